# revision 28
# baseline (speedup 1.0000x reference)
"""Trainium2 Bass kernel for nn_CoreBlock (circulant attention + 2-layer FFN).

Contract: kernel(**inputs) takes FULL unsharded inputs (as produced by
setup_inputs) and returns the FULL [16, 1024, 768] f32 output.

Strategy: pure data-parallel over batch - 8 NeuronCores x 2 batches each.
All weights replicated. Per core:
  phase A: LayerNorm(x) -> u (gpsimd), PE-transpose u, v = u_dt.T @ Wv' per
           token-chunk; results land in a resident V tensor in SBUF.
  phase B: per head h: circulant matmul using an 8-tile Toeplitz bank
           T[h,m]; both batches fused into one 128-wide moving operand and
           multi-chunk (<=512 free) moving slices, so the whole head is
           ~22 large matmuls instead of 128 tiny ones. Residual-added into
           X (X becomes x1 = x + y).
  phase C: 2x [Dense -> LayerNorm -> swish], software-pipelined in groups
           of 4 token chunks. rstd comes from a DVE Newton rsqrt (bit-hack
           seed + 2 iterations) so the scalar engine never loads the Sqrt
           table and the Silu table stays resident for the whole phase.
  tail:    log_cosh(w) = softplus(2w) - w - ln2 (one ACT pass, no abs),
           group-batched output DMA.

Matmul operands are bf16 (full-rate PE, fp32 PSUM accumulation); stats and
elementwise math fp32. PSUM->SBUF copies are split between the scalar and
vector engines to balance them; gpsimd takes pure-SBUF elementwise work.
"""

import math
import numpy as np
import ml_dtypes

import concourse.bass as bass
import concourse.tile as tile
from concourse import bacc, mybir
from concourse.bass_utils import run_bass_kernel_spmd

BF16 = ml_dtypes.bfloat16

B, N, D = 16, 1024, 768
H, HS, L = 12, 64, 2
EPS = 1e-6
NCORES = 8
BPC = B // NCORES          # batches per core
NJ = N // 128              # token chunks per batch (8)
NT = BPC * NJ              # token chunks per core (16)
DC = D // 128              # feature chunks (6)
AB = 4                     # chunks per pipeline group

F32 = mybir.dt.float32
I32 = mybir.dt.int32
BF = mybir.dt.bfloat16
Alu = mybir.AluOpType
Act = mybir.ActivationFunctionType

LN2 = math.log(2.0)
# fp32 whose bit pattern is 0x5f3759df (fast-rsqrt magic constant)
MAGIC_F = float(np.int32(0x5F3759DF).view(np.float32))

USE_SOFTPLUS = False       # no Softplus table on TRN2; use Exp+Ln log_cosh
USE_GPSIMD = False         # gpsimd tensor ops measured ~20x slower than DVE

TRACE = False              # test harness sets this for profiling runs
TRACE_KW = {}

_cache = {}


class _Bacc(bacc.Bacc):
    """Bacc whose activation-table chooser sees Exp and Ln only in the
    combined natural_log_exp_and_others set, so the log_cosh tail needs one
    table load instead of alternating exp/ln loads. List order (and thus
    act_func_set_id) is unchanged; only the per-set membership used for
    choosing is filtered."""

    def insert_act_table_loads(self):
        has_activation = any(
            isinstance(i, mybir.InstActivation)
            for b in self.main_func.blocks
            for i in b.instructions
        )
        if not has_activation:
            return
        from concourse.hw_specs import get_activation_tables
        Fn = mybir.ActivationFunctionType
        tables = []
        for name, fns in get_activation_tables(self.m.arch).items():
            if name != "natural_log_exp_and_others":
                fns = fns - {Fn.Exp, Fn.Ln}
            tables.append((name, fns))
        import concourse._compat as _compat  # noqa: F401
        from concourse.bacc import _bass_rust
        _bass_rust.insert_act_table_loads(self, tables)


def _build(cv_nonzero, bf_nonzero, lnf_uniform):
    nc = _Bacc("TRN2", target_bir_lowering=False, debug=False)

    xs = nc.dram_tensor("xs", (BPC, N, D), F32, kind="ExternalInput").ap()
    wv = nc.dram_tensor("wv", (D, D), BF, kind="ExternalInput").ap()
    wf = nc.dram_tensor("wf", (L, D, D), BF, kind="ExternalInput").ap()
    tb_d = nc.dram_tensor("tbank", (H, 128, NJ * 128), BF, kind="ExternalInput").ap()
    id32 = nc.dram_tensor("id32", (128, 128), F32, kind="ExternalInput").ap()
    idbf = nc.dram_tensor("idbf", (128, 128), BF, kind="ExternalInput").ap()
    cv_d = nc.dram_tensor("cv", (D,), F32, kind="ExternalInput").ap()
    bf_d = nc.dram_tensor("bfb", (L, D), F32, kind="ExternalInput").ap()
    lnfs_d = nc.dram_tensor("lnfs", (L, D), F32, kind="ExternalInput").ap()
    lnfb_d = nc.dram_tensor("lnfb", (L, D), F32, kind="ExternalInput").ap()
    out_d = nc.dram_tensor("out", (BPC, N, D), F32, kind="ExternalOutput").ap()

    with tile.TileContext(nc) as tc:
        _emit(nc, tc, xs, wv, wf, tb_d, id32, idbf, cv_d, bf_d, lnfs_d, lnfb_d,
              out_d, cv_nonzero, bf_nonzero, lnf_uniform)
    nc.compile()
    return nc


def _newton_rsqrt(nc, pool, dst, var_ap, magict, g):
    """dst[128, g] = 1/sqrt(var + EPS) via bit-hack seed + 2 Newton steps.
    All on the vector engine; no activation tables involved."""
    vv = pool.tile([128, 16], F32, tag="nvv", name="nvv")[:, 0:g]
    nc.vector.tensor_scalar(vv, var_ap, EPS, None, op0=Alu.add)
    y0 = pool.tile([128, 16], F32, tag="ny0", name="ny0")[:, 0:g]
    nc.vector.tensor_scalar(y0.bitcast(I32), vv.bitcast(I32), 1, None,
                            op0=Alu.logical_shift_right)
    nc.vector.tensor_tensor(y0.bitcast(I32), magict[:, 0:g].bitcast(I32),
                            y0.bitcast(I32), op=Alu.subtract)
    t1 = pool.tile([128, 16], F32, tag="nt1", name="nt1")[:, 0:g]
    for it in range(2):
        nc.vector.tensor_tensor(t1, y0, y0, op=Alu.mult)
        nc.vector.tensor_tensor(t1, t1, vv, op=Alu.mult)
        nc.vector.tensor_scalar(t1, t1, -0.5, 1.5, op0=Alu.mult, op1=Alu.add)
        nc.vector.tensor_tensor(dst if it == 1 else y0, y0, t1, op=Alu.mult)


def _emit(nc, tc, xs, wv, wf, tb_d, id32, idbf, cv_d, bf_d, lnfs_d, lnfb_d,
          out_d, cv_nonzero, bf_nonzero, lnf_uniform):
    from contextlib import ExitStack
    gps = nc.gpsimd if USE_GPSIMD else nc.vector
    ctx = ExitStack()
    with ctx:
        consts = ctx.enter_context(tc.tile_pool(name="consts", bufs=1))
        xpool = ctx.enter_context(tc.tile_pool(name="xpool", bufs=1))
        vpool = ctx.enter_context(tc.tile_pool(name="vpool", bufs=1))
        acts = ctx.enter_context(tc.tile_pool(name="acts", bufs=18))
        upool = ctx.enter_context(tc.tile_pool(name="upool", bufs=3))
        x1p = ctx.enter_context(tc.tile_pool(name="x1p", bufs=3))
        dtp = ctx.enter_context(tc.tile_pool(name="dtp", bufs=3))
        stat = ctx.enter_context(tc.tile_pool(name="stat", bufs=4))
        statp = ctx.enter_context(tc.tile_pool(name="statp", bufs=2))
        scrp = ctx.enter_context(tc.tile_pool(name="scrp", bufs=2))
        wkp = ctx.enter_context(tc.tile_pool(name="wkp", bufs=2))
        ps_mm = ctx.enter_context(tc.tile_pool(name="ps_mm", bufs=4, space="PSUM"))

        # ---- constants ----
        wv_s = consts.tile([128, DC, D], BF, tag="wv")
        wf_s = consts.tile([128, L, DC, D], BF, tag="wf")
        tb_s = consts.tile([128, H, NJ, 128], BF, tag="tb")
        magict = consts.tile([128, 16], F32, tag="magic")
        nc.vector.memset(magict[:], MAGIC_F)
        onet = consts.tile([128, 1], F32, tag="one")
        nc.vector.memset(onet[:], 1.0)
        zerot = consts.tile([128, 1], F32, tag="zero")
        nc.vector.memset(zerot[:], 0.0)

        # weight/constant DMAs dispatch from the (otherwise idle) gpsimd
        # queue so they don't serialize behind the x-chunk dispatches on
        # the sync engine.
        nc.gpsimd.dma_start(wv_s[:], wv.rearrange("(c p) f -> p c f", p=128))

        cvt = None
        if cv_nonzero:
            cvt = consts.tile([128, D], F32, tag="cv")
            nc.gpsimd.dma_start(cvt[:], cv_d.to_broadcast((128, D)))
        bft = [None] * L
        lnfst = [None] * L
        lnfbt = [None] * L
        for l in range(L):
            if bf_nonzero[l]:
                bft[l] = consts.tile([128, D], F32, tag=f"bf{l}")
                nc.gpsimd.dma_start(bft[l][:], bf_d[l].to_broadcast((128, D)))
            if lnf_uniform[l] is None:
                lnfst[l] = consts.tile([128, D], F32, tag=f"lnfs{l}")
                nc.gpsimd.dma_start(lnfst[l][:], lnfs_d[l].to_broadcast((128, D)))
                lnfbt[l] = consts.tile([128, D], F32, tag=f"lnfb{l}")
                nc.gpsimd.dma_start(lnfbt[l][:], lnfb_d[l].to_broadcast((128, D)))

        # ---- resident tensors ----
        X = xpool.tile([128, BPC, NJ, D], F32, tag="X")         # x, then x1
        V = vpool.tile([128, H, NJ, BPC, HS], BF, tag="V")      # per-head values

        # ================= phase A: LN + v-projection =================
        mvA = statp.tile([128, NT, 2], F32, tag="mvA")
        rsA = statp.tile([128, NT], F32, tag="rsA")
        # all x-chunk DMAs dispatch up front (no waits -> the sync queue
        # never head-of-line blocks the later XBAR transposes behind them)
        for t in range(NT):
            b, jc = divmod(t, NJ)
            nc.sync.dma_start(X[:, b, jc, :],
                              xs[b, jc * 128:(jc + 1) * 128, :])
        for t0 in range(0, NT, AB):
            for t in range(t0, t0 + AB):
                b, jc = divmod(t, NJ)
                xt = X[:, b, jc, :]
                st = stat.tile([128, 2, 6], F32, tag="bst")
                nc.vector.bn_stats(st[:, 0, :], xt[:, 0:512])
                nc.vector.bn_stats(st[:, 1, :], xt[:, 512:D])
                nc.vector.bn_aggr(mvA[:, t, :], st[:])
            _newton_rsqrt(nc, stat, rsA[:, t0:t0 + AB], mvA[:, t0:t0 + AB, 1],
                          magict, AB)
            if t0 == 0:
                nc.gpsimd.dma_start(
                    tb_s[:], tb_d.rearrange("h p (m f) -> p h m f", m=NJ))
            for t in range(t0, t0 + AB):
                b, jc = divmod(t, NJ)
                xt = X[:, b, jc, :]
                u = upool.tile([128, D], BF, tag="u")
                gps.tensor_scalar(u[:], xt, mvA[:, t, 0:1], rsA[:, t:t + 1],
                                  op0=Alu.subtract, op1=Alu.mult)
                # XBAR crossbar transpose on the DMA path: replaces 6 PE
                # transposes + a PSUM->SBUF copy per chunk.
                udt = dtp.tile([128, DC, 128], BF, tag="udt")
                nc.sync.dma_start_transpose(udt[:], u[:])
                pv = ps_mm.tile([128, NJ, 128], F32, tag="mm")
                pvf = pv[:].rearrange("p a b -> p (a b)")
                for c in range(DC):
                    nc.tensor.matmul(pvf[:, 0:512], udt[:, c, :],
                                     wv_s[:, c, 0:512],
                                     start=(c == 0), stop=(c == DC - 1))
                    nc.tensor.matmul(pvf[:, 512:D], udt[:, c, :],
                                     wv_s[:, c, 512:D],
                                     start=(c == 0), stop=(c == DC - 1))
                pv3 = pvf[:, 0:D].rearrange("p (h k) -> p h k", h=H)
                if cv_nonzero:
                    cv3 = cvt[:].rearrange("p (h k) -> p h k", h=H)
                    nc.vector.tensor_tensor(V[:, :, jc, b, :], pv3, cv3,
                                            op=Alu.add)
                else:
                    # DVE is phase A's critical engine; the cast goes to ACT
                    nc.scalar.copy(V[:, :, jc, b, :], pv3)
            if t0 == AB:
                nc.gpsimd.dma_start(wf_s[:], wf.rearrange("l (c p) f -> p l c f", p=128))

        # ================= phase B: circulant attention =================
        # y[ic] = sum_m T[m] @ V[(ic+m) % NJ], both batches fused in the
        # moving operand (free = jc-run * BPC*HS, up to 512).
        for h in range(H):
            pc = ps_mm.tile([128, NJ, BPC * HS], F32, tag="mm")
            for m in range(NJ):
                for ic0, jc0, ln in ((0, m, NJ - m), (NJ - m, 0, m)):
                    p0 = 0
                    while p0 < ln:
                        pl = min(4, ln - p0)
                        nc.tensor.matmul(
                            pc[:, ic0 + p0:ic0 + p0 + pl, :],
                            tb_s[:, h, m, :],
                            V[:, h, jc0 + p0:jc0 + p0 + pl, :, :],
                            start=(m == 0), stop=(m == NJ - 1),
                            skip_group_check=True)
                        p0 += pl
            for b in range(BPC):
                xap = X[:, b, :, h * HS:(h + 1) * HS]
                nc.vector.tensor_tensor(xap, xap, pc[:, :, b * HS:(b + 1) * HS],
                                        op=Alu.add)

        # ================= phase C + tail, in two half-batches =================
        # Each half (8 chunks = one batch) runs L1 -> L2 -> log_cosh tail;
        # the second half's matmuls overlap the first half's scalar/vector
        # tail so the PE never sits idle for long.
        inv_d = 1.0 / D
        zcur = [None] * NT
        # layer-2 outputs land in one resident tensor (reusing the Toeplitz
        # bank's SBUF slot, dead after phase B) so the tail can fence on
        # a whole half at once; the bf16 exp buffer reuses V's slot.
        Z2 = consts.tile([128, NT, D], BF, tag="tb", name="Z2")
        awl = vpool.tile([128, NT, D], BF, tag="V")
        stats_t = {}
        for l in range(L):
            stats_t[l] = dict(
                sums=statp.tile([128, NT], F32, tag=f"sum{l}", name="sums"),
                ssq=statp.tile([128, NT], F32, tag=f"ssq{l}", name="ssq"),
                muA=statp.tile([128, NT], F32, tag=f"mu{l}", name="muA"),
                rsF=statp.tile([128, NT], F32, tag=f"rs{l}", name="rsF"),
                biasF=statp.tile([128, NT], F32, tag=f"bi{l}", name="biasF"),
            )
        fence = statp.tile([128, 2], F32, tag="fence")

        for l in range(L):
                fast = lnf_uniform[l] is not None
                stt = stats_t[l]
                sums, ssq = stt["sums"], stt["ssq"]
                muA, rsF, biasF = stt["muA"], stt["rsF"], stt["biasF"]
                for g0 in range(0, NT, AB):
                    for t in range(g0, g0 + AB):
                        b, jc = divmod(t, NJ)
                        if l == 0:
                            # bf16 copy of x1 feeds the 2-byte XBAR transpose
                            src = x1p.tile([128, D], BF, tag="x1b", name="x1b")
                            nc.vector.tensor_copy(src[:], X[:, b, jc, :])
                            src = src[:]
                        else:
                            src = zcur[t][:]
                        zdt = dtp.tile([128, DC, 128], BF, tag="udt")
                        nc.sync.dma_start_transpose(zdt[:], src)
                        pf = ps_mm.tile([128, NJ, 128], F32, tag="mm")
                        pff = pf[:].rearrange("p a b -> p (a b)")
                        for c in range(DC):
                            nc.tensor.matmul(pff[:, 0:512],
                                             zdt[:, c, :],
                                             wf_s[:, l, c, 0:512],
                                             start=(c == 0), stop=(c == DC - 1))
                            nc.tensor.matmul(pff[:, 512:D],
                                             zdt[:, c, :],
                                             wf_s[:, l, c, 512:D],
                                             start=(c == 0), stop=(c == DC - 1))
                        if bf_nonzero[l]:
                            nc.vector.tensor_tensor(pff[:, 0:D], pff[:, 0:D],
                                                    bft[l][:], op=Alu.add)
                        y = acts.tile([128, D], BF, tag="acts")
                        nc.scalar.activation(y[:], pff[:, 0:D], Act.Copy,
                                             accum_out=sums[:, t:t + 1])
                        scr = scrp.tile([128, D], BF, tag="scr")
                        nc.vector.scalar_tensor_tensor(
                            scr[:], y[:], 0.0, y[:], op0=Alu.add, op1=Alu.mult,
                            accum_out=ssq[:, t:t + 1])
                        zcur[t] = y
                    # group epilogue: var -> rstd (DVE Newton) -> Silu
                    g = slice(g0, g0 + AB)
                    nc.vector.tensor_scalar(muA[:, g], sums[:, g], inv_d, None,
                                            op0=Alu.mult)
                    m2 = stat.tile([128, 16], F32, tag="m2", name="m2")[:, 0:AB]
                    nc.vector.tensor_scalar(m2, ssq[:, g], inv_d, None,
                                            op0=Alu.mult)
                    var = stat.tile([128, 16], F32, tag="var", name="var")[:, 0:AB]
                    nc.vector.scalar_tensor_tensor(var, muA[:, g], -1.0,
                                                   muA[:, g], op0=Alu.mult,
                                                   op1=Alu.mult)
                    nc.vector.tensor_tensor(var, m2, var, op=Alu.add)
                    _newton_rsqrt(nc, stat, rsF[:, g], var, magict, AB)
                    if fast:
                        cs, cb = lnf_uniform[l]
                        if cs != 1.0:
                            nc.vector.tensor_scalar(rsF[:, g], rsF[:, g],
                                                    float(cs), None,
                                                    op0=Alu.mult)
                        nc.vector.scalar_tensor_tensor(biasF[:, g], muA[:, g],
                                                       -1.0, rsF[:, g],
                                                       op0=Alu.mult,
                                                       op1=Alu.mult)
                        if cb != 0.0:
                            nc.vector.tensor_scalar(biasF[:, g], biasF[:, g],
                                                    float(cb), None,
                                                    op0=Alu.add)
                        for t in range(g0, g0 + AB):
                            y = zcur[t]
                            dst = y[:] if l == 0 else Z2[:, t, :]
                            nc.scalar.activation(dst, y[:], Act.Silu,
                                                 bias=biasF[:, t:t + 1],
                                                 scale=rsF[:, t:t + 1])
                            if l == 1:
                                zcur[t] = None
                    else:
                        for t in range(g0, g0 + AB):
                            y = zcur[t]
                            tmp = acts.tile([128, D], BF, tag="acts")
                            nc.vector.tensor_scalar(tmp[:], y[:],
                                                    muA[:, t:t + 1],
                                                    rsF[:, t:t + 1],
                                                    op0=Alu.subtract,
                                                    op1=Alu.mult)
                            nc.vector.tensor_tensor(tmp[:], tmp[:], lnfst[l][:],
                                                    op=Alu.mult)
                            dst = tmp[:] if l == 0 else Z2[:, t, :]
                            nc.vector.tensor_tensor(dst, tmp[:], lnfbt[l][:],
                                                    op=Alu.add)
                            nc.scalar.activation(dst, dst, Act.Silu,
                                                 bias=zerot[:])
                            zcur[t] = tmp if l == 0 else None

        # ---- tail, in two halves: log_cosh(w) = |w| + log1p(exp(-2|w|)) - ln2
        # Half 1's exp/ln overlaps layer 2's back half on the PE; per-chunk
        # output DMAs overlap the transfers with the remaining compute.
        for h0 in range(0, NT, NJ):
            hh = h0 // NJ
            for t in range(h0, h0 + NJ):
                b, jc = divmod(t, NJ)
                xt = X[:, b, jc, :]
                nc.vector.tensor_tensor(xt, xt, Z2[:, t, :], op=Alu.add)
                nc.vector.scalar_tensor_tensor(xt, xt, -1.0, xt,
                                               op0=Alu.mult, op1=Alu.max)
            # fence: a zero [128,1] that depends on every layer-2 Silu of
            # this half; used as the Exp bias so the scheduler cannot
            # interleave tail Exps between Silus (activation-table thrash).
            fscr = stat.tile([128, NJ], F32, tag="fscr", name="fscr")
            nc.vector.tensor_scalar(fscr[:], Z2[:, h0:h0 + NJ, 0], 0.0, 0.0,
                                    op0=Alu.mult, op1=Alu.mult,
                                    accum_out=fence[:, hh:hh + 1])
            for t in range(h0, h0 + NJ):
                b, jc = divmod(t, NJ)
                nc.scalar.activation(awl[:, t, :], X[:, b, jc, :], Act.Exp,
                                     bias=fence[:, hh:hh + 1], scale=-2.0)
            for t in range(h0, h0 + NJ):
                nc.scalar.activation(awl[:, t, :], awl[:, t, :], Act.Ln,
                                     bias=onet[:], scale=1.0)
            for t in range(h0, h0 + NJ):
                b, jc = divmod(t, NJ)
                sp = wkp.tile([128, D], F32, tag="sp", name="sp", bufs=4)
                nc.vector.scalar_tensor_tensor(sp[:], awl[:, t, :],
                                               -LN2, X[:, b, jc, :],
                                               op0=Alu.add, op1=Alu.add)
                nc.scalar.dma_start(out_d[b, jc * 128:(jc + 1) * 128, :], sp[:])


def _prep(inputs):
    x = np.asarray(inputs["x"], np.float32)
    ln1_s = np.asarray(inputs["ln1_scale"], np.float32)
    ln1_b = np.asarray(inputs["ln1_bias"], np.float32)
    Wv = np.asarray(inputs["Wv"], np.float32)
    alpha = np.asarray(inputs["alpha"], np.float32)
    Wf = np.asarray(inputs["Wf"], np.float32)
    bfv = np.asarray(inputs["bf"], np.float32)
    lnf_s = np.asarray(inputs["lnf_scale"], np.float32)
    lnf_b = np.asarray(inputs["lnf_bias"], np.float32)

    Wv_flat = Wv.transpose(1, 0, 2).reshape(D, H * HS)
    Wvp = (ln1_s[:, None] * Wv_flat).astype(BF16)
    cv = (ln1_b @ Wv_flat).astype(np.float32)

    ar = alpha[:, (-np.arange(N)) % N]
    ar2 = np.concatenate([ar, ar], axis=1)
    m_ = np.arange(NJ)[:, None, None]
    p_ = np.arange(128)[None, :, None]
    f_ = np.arange(128)[None, None, :]
    T = ar2[:, N + 128 * m_ + p_ - f_]                  # [H, NJ, 128, 128]
    tbank = np.ascontiguousarray(
        T.transpose(0, 2, 1, 3).reshape(H, 128, NJ * 128)).astype(BF16)

    cv_nonzero = bool(np.any(cv))
    bf_nonzero = tuple(bool(np.any(bfv[l])) for l in range(L))
    lnf_uniform = []
    for l in range(L):
        s, bb = lnf_s[l], lnf_b[l]
        if np.all(s == s[0]) and np.all(bb == bb[0]):
            lnf_uniform.append((float(s[0]), float(bb[0])))
        else:
            lnf_uniform.append(None)
    key = (cv_nonzero, bf_nonzero, tuple(lnf_uniform))

    common = {
        "wv": np.ascontiguousarray(Wvp),
        "wf": Wf.astype(BF16),
        "tbank": tbank,
        "id32": np.eye(128, dtype=np.float32),
        "idbf": np.eye(128, dtype=BF16),
        "cv": cv,
        "bfb": bfv,
        "lnfs": lnf_s,
        "lnfb": lnf_b,
    }
    return x, key, common, (cv_nonzero, bf_nonzero, lnf_uniform)


def kernel(**inputs):
    x, key, common, flags = _prep(inputs)
    if key not in _cache:
        _cache[key] = _build(*flags)
    nc = _cache[key]
    in_maps = []
    for i in range(NCORES):
        m = dict(common)
        m["xs"] = np.ascontiguousarray(x[i * BPC:(i + 1) * BPC])
        in_maps.append(m)
    res = run_bass_kernel_spmd(nc, in_maps, core_ids=list(range(NCORES)),
                               trace=TRACE, **TRACE_KW)
    kernel.last_result = res
    out = np.empty((B, N, D), np.float32)
    for i in range(NCORES):
        out[i * BPC:(i + 1) * BPC] = res.results[i]["out"]
    return out


# revision 30
# speedup vs baseline: 1.0076x; 1.0076x over previous
"""Trainium2 Bass kernel for nn_CoreBlock (circulant attention + 2-layer FFN).

Contract: kernel(**inputs) takes FULL unsharded inputs (as produced by
setup_inputs) and returns the FULL [16, 1024, 768] f32 output.

Strategy: pure data-parallel over batch - 8 NeuronCores x 2 batches each.
All weights replicated. Per core:
  phase A: LayerNorm(x) -> u (gpsimd), PE-transpose u, v = u_dt.T @ Wv' per
           token-chunk; results land in a resident V tensor in SBUF.
  phase B: per head h: circulant matmul using an 8-tile Toeplitz bank
           T[h,m]; both batches fused into one 128-wide moving operand and
           multi-chunk (<=512 free) moving slices, so the whole head is
           ~22 large matmuls instead of 128 tiny ones. Residual-added into
           X (X becomes x1 = x + y).
  phase C: 2x [Dense -> LayerNorm -> swish], software-pipelined in groups
           of 4 token chunks. rstd comes from a DVE Newton rsqrt (bit-hack
           seed + 2 iterations) so the scalar engine never loads the Sqrt
           table and the Silu table stays resident for the whole phase.
  tail:    log_cosh(w) = softplus(2w) - w - ln2 (one ACT pass, no abs),
           group-batched output DMA.

Matmul operands are bf16 (full-rate PE, fp32 PSUM accumulation); stats and
elementwise math fp32. PSUM->SBUF copies are split between the scalar and
vector engines to balance them; gpsimd takes pure-SBUF elementwise work.
"""

import math
import numpy as np
import ml_dtypes

import concourse.bass as bass
import concourse.tile as tile
from concourse import bacc, mybir
from concourse.bass_utils import run_bass_kernel_spmd

BF16 = ml_dtypes.bfloat16

B, N, D = 16, 1024, 768
H, HS, L = 12, 64, 2
EPS = 1e-6
NCORES = 8
BPC = B // NCORES          # batches per core
NJ = N // 128              # token chunks per batch (8)
NT = BPC * NJ              # token chunks per core (16)
DC = D // 128              # feature chunks (6)
AB = 4                     # chunks per pipeline group

F32 = mybir.dt.float32
I32 = mybir.dt.int32
BF = mybir.dt.bfloat16
Alu = mybir.AluOpType
Act = mybir.ActivationFunctionType

LN2 = math.log(2.0)
# fp32 whose bit pattern is 0x5f3759df (fast-rsqrt magic constant)
MAGIC_F = float(np.int32(0x5F3759DF).view(np.float32))

USE_SOFTPLUS = False       # no Softplus table on TRN2; use Exp+Ln log_cosh
USE_GPSIMD = False         # gpsimd tensor ops measured ~20x slower than DVE

TRACE = False              # test harness sets this for profiling runs
TRACE_KW = {}

_cache = {}


class _Bacc(bacc.Bacc):
    """Bacc whose activation-table chooser sees Exp and Ln only in the
    combined natural_log_exp_and_others set, so the log_cosh tail needs one
    table load instead of alternating exp/ln loads. List order (and thus
    act_func_set_id) is unchanged; only the per-set membership used for
    choosing is filtered."""

    def insert_act_table_loads(self):
        has_activation = any(
            isinstance(i, mybir.InstActivation)
            for b in self.main_func.blocks
            for i in b.instructions
        )
        if not has_activation:
            return
        from concourse.hw_specs import get_activation_tables
        Fn = mybir.ActivationFunctionType
        tables = []
        for name, fns in get_activation_tables(self.m.arch).items():
            if name != "natural_log_exp_and_others":
                fns = fns - {Fn.Exp, Fn.Ln}
            tables.append((name, fns))
        import concourse._compat as _compat  # noqa: F401
        from concourse.bacc import _bass_rust
        _bass_rust.insert_act_table_loads(self, tables)


def _build(cv_nonzero, bf_nonzero, lnf_uniform):
    nc = _Bacc("TRN2", target_bir_lowering=False, debug=False)

    xs = nc.dram_tensor("xs", (BPC, N, D), F32, kind="ExternalInput").ap()
    wv = nc.dram_tensor("wv", (D, D), BF, kind="ExternalInput").ap()
    wf = nc.dram_tensor("wf", (L, D, D), BF, kind="ExternalInput").ap()
    tb_d = nc.dram_tensor("tbank", (H, 128, NJ * 128), BF, kind="ExternalInput").ap()
    id32 = nc.dram_tensor("id32", (128, 128), F32, kind="ExternalInput").ap()
    idbf = nc.dram_tensor("idbf", (128, 128), BF, kind="ExternalInput").ap()
    cv_d = nc.dram_tensor("cv", (D,), F32, kind="ExternalInput").ap()
    bf_d = nc.dram_tensor("bfb", (L, D), F32, kind="ExternalInput").ap()
    lnfs_d = nc.dram_tensor("lnfs", (L, D), F32, kind="ExternalInput").ap()
    lnfb_d = nc.dram_tensor("lnfb", (L, D), F32, kind="ExternalInput").ap()
    out_d = nc.dram_tensor("out", (BPC, N, D), F32, kind="ExternalOutput").ap()

    with tile.TileContext(nc) as tc:
        _emit(nc, tc, xs, wv, wf, tb_d, id32, idbf, cv_d, bf_d, lnfs_d, lnfb_d,
              out_d, cv_nonzero, bf_nonzero, lnf_uniform)
    nc.compile()
    return nc


def _newton_rsqrt(nc, pool, dst, var_ap, magict, g):
    """dst[128, g] = 1/sqrt(var + EPS) via bit-hack seed + 2 Newton steps.
    All on the vector engine; no activation tables involved."""
    vv = pool.tile([128, 16], F32, tag="nvv", name="nvv")[:, 0:g]
    nc.vector.tensor_scalar(vv, var_ap, EPS, None, op0=Alu.add)
    y0 = pool.tile([128, 16], F32, tag="ny0", name="ny0")[:, 0:g]
    nc.vector.tensor_scalar(y0.bitcast(I32), vv.bitcast(I32), 1, None,
                            op0=Alu.logical_shift_right)
    nc.vector.tensor_tensor(y0.bitcast(I32), magict[:, 0:g].bitcast(I32),
                            y0.bitcast(I32), op=Alu.subtract)
    t1 = pool.tile([128, 16], F32, tag="nt1", name="nt1")[:, 0:g]
    for it in range(2):
        nc.vector.tensor_tensor(t1, y0, y0, op=Alu.mult)
        nc.vector.tensor_tensor(t1, t1, vv, op=Alu.mult)
        nc.vector.tensor_scalar(t1, t1, -0.5, 1.5, op0=Alu.mult, op1=Alu.add)
        nc.vector.tensor_tensor(dst if it == 1 else y0, y0, t1, op=Alu.mult)


def _emit(nc, tc, xs, wv, wf, tb_d, id32, idbf, cv_d, bf_d, lnfs_d, lnfb_d,
          out_d, cv_nonzero, bf_nonzero, lnf_uniform):
    from contextlib import ExitStack
    gps = nc.gpsimd if USE_GPSIMD else nc.vector
    ctx = ExitStack()
    with ctx:
        consts = ctx.enter_context(tc.tile_pool(name="consts", bufs=1))
        xpool = ctx.enter_context(tc.tile_pool(name="xpool", bufs=1))
        vpool = ctx.enter_context(tc.tile_pool(name="vpool", bufs=1))
        acts = ctx.enter_context(tc.tile_pool(name="acts", bufs=18))
        upool = ctx.enter_context(tc.tile_pool(name="upool", bufs=3))
        x1p = ctx.enter_context(tc.tile_pool(name="x1p", bufs=3))
        dtp = ctx.enter_context(tc.tile_pool(name="dtp", bufs=3))
        stat = ctx.enter_context(tc.tile_pool(name="stat", bufs=4))
        statp = ctx.enter_context(tc.tile_pool(name="statp", bufs=2))
        scrp = ctx.enter_context(tc.tile_pool(name="scrp", bufs=2))
        wkp = ctx.enter_context(tc.tile_pool(name="wkp", bufs=2))
        ps_mm = ctx.enter_context(tc.tile_pool(name="ps_mm", bufs=4, space="PSUM"))

        # ---- constants ----
        wv_s = consts.tile([128, DC, D], BF, tag="wv")
        wf_s = consts.tile([128, L, DC, D], BF, tag="wf")
        tb_s = consts.tile([128, H, NJ, 128], BF, tag="tb")
        magict = consts.tile([128, 16], F32, tag="magic")
        nc.vector.memset(magict[:], MAGIC_F)
        onet = consts.tile([128, 1], F32, tag="one")
        nc.vector.memset(onet[:], 1.0)
        zerot = consts.tile([128, 1], F32, tag="zero")
        nc.vector.memset(zerot[:], 0.0)

        # weight/constant DMAs go through the gpsimd software-DGE queue (a
        # third parallel DMA path); wv is split per 128-block so the first
        # matmuls can start before the whole tensor lands.
        wv_r = wv.rearrange("(c p) f -> p c f", p=128)
        for c in range(DC):
            nc.gpsimd.dma_start(wv_s[:, c, :], wv_r[:, c, :])

        cvt = None
        if cv_nonzero:
            cvt = consts.tile([128, D], F32, tag="cv")
            nc.gpsimd.dma_start(cvt[:], cv_d.to_broadcast((128, D)))
        bft = [None] * L
        lnfst = [None] * L
        lnfbt = [None] * L
        for l in range(L):
            if bf_nonzero[l]:
                bft[l] = consts.tile([128, D], F32, tag=f"bf{l}")
                nc.gpsimd.dma_start(bft[l][:], bf_d[l].to_broadcast((128, D)))
            if lnf_uniform[l] is None:
                lnfst[l] = consts.tile([128, D], F32, tag=f"lnfs{l}")
                nc.gpsimd.dma_start(lnfst[l][:], lnfs_d[l].to_broadcast((128, D)))
                lnfbt[l] = consts.tile([128, D], F32, tag=f"lnfb{l}")
                nc.gpsimd.dma_start(lnfbt[l][:], lnfb_d[l].to_broadcast((128, D)))

        # ---- resident tensors ----
        X = xpool.tile([128, BPC, NJ, D], F32, tag="X")         # x, then x1
        V = vpool.tile([128, H, NJ, BPC, HS], BF, tag="V")      # per-head values

        # ================= phase A: LN + v-projection =================
        mvA = statp.tile([128, NT, 2], F32, tag="mvA")
        rsA = statp.tile([128, NT], F32, tag="rsA")
        # all x-chunk DMAs dispatch up front, alternating between the two
        # hardware DGE paths (sync / scalar) so the 16 transfers stream on
        # two queues in parallel instead of serializing on one
        for t in range(NT):
            b, jc = divmod(t, NJ)
            eng = nc.sync if t % 2 == 0 else nc.scalar
            eng.dma_start(X[:, b, jc, :],
                          xs[b, jc * 128:(jc + 1) * 128, :])
        for t0 in range(0, NT, AB):
            for t in range(t0, t0 + AB):
                b, jc = divmod(t, NJ)
                xt = X[:, b, jc, :]
                st = stat.tile([128, 2, 6], F32, tag="bst")
                nc.vector.bn_stats(st[:, 0, :], xt[:, 0:512])
                nc.vector.bn_stats(st[:, 1, :], xt[:, 512:D])
                nc.vector.bn_aggr(mvA[:, t, :], st[:])
            _newton_rsqrt(nc, stat, rsA[:, t0:t0 + AB], mvA[:, t0:t0 + AB, 1],
                          magict, AB)
            if t0 == 0:
                nc.gpsimd.dma_start(
                    tb_s[:], tb_d.rearrange("h p (m f) -> p h m f", m=NJ))
            for t in range(t0, t0 + AB):
                b, jc = divmod(t, NJ)
                xt = X[:, b, jc, :]
                u = upool.tile([128, D], BF, tag="u")
                gps.tensor_scalar(u[:], xt, mvA[:, t, 0:1], rsA[:, t:t + 1],
                                  op0=Alu.subtract, op1=Alu.mult)
                # XBAR crossbar transpose on the DMA path: replaces 6 PE
                # transposes + a PSUM->SBUF copy per chunk.
                udt = dtp.tile([128, DC, 128], BF, tag="udt")
                nc.sync.dma_start_transpose(udt[:], u[:])
                pv = ps_mm.tile([128, NJ, 128], F32, tag="mm")
                pvf = pv[:].rearrange("p a b -> p (a b)")
                for c in range(DC):
                    nc.tensor.matmul(pvf[:, 0:512], udt[:, c, :],
                                     wv_s[:, c, 0:512],
                                     start=(c == 0), stop=(c == DC - 1))
                    nc.tensor.matmul(pvf[:, 512:D], udt[:, c, :],
                                     wv_s[:, c, 512:D],
                                     start=(c == 0), stop=(c == DC - 1))
                pv3 = pvf[:, 0:D].rearrange("p (h k) -> p h k", h=H)
                if cv_nonzero:
                    cv3 = cvt[:].rearrange("p (h k) -> p h k", h=H)
                    nc.vector.tensor_tensor(V[:, :, jc, b, :], pv3, cv3,
                                            op=Alu.add)
                else:
                    # DVE is phase A's critical engine; the cast goes to ACT
                    nc.scalar.copy(V[:, :, jc, b, :], pv3)
            if t0 == AB:
                nc.gpsimd.dma_start(wf_s[:], wf.rearrange("l (c p) f -> p l c f", p=128))

        # ================= phase B: circulant attention =================
        # y[ic] = sum_m T[m] @ V[(ic+m) % NJ], both batches fused in the
        # moving operand (free = jc-run * BPC*HS, up to 512).
        for h in range(H):
            pc = ps_mm.tile([128, NJ, BPC * HS], F32, tag="mm")
            for m in range(NJ):
                for ic0, jc0, ln in ((0, m, NJ - m), (NJ - m, 0, m)):
                    p0 = 0
                    while p0 < ln:
                        pl = min(4, ln - p0)
                        nc.tensor.matmul(
                            pc[:, ic0 + p0:ic0 + p0 + pl, :],
                            tb_s[:, h, m, :],
                            V[:, h, jc0 + p0:jc0 + p0 + pl, :, :],
                            start=(m == 0), stop=(m == NJ - 1),
                            skip_group_check=True)
                        p0 += pl
            for b in range(BPC):
                xap = X[:, b, :, h * HS:(h + 1) * HS]
                nc.vector.tensor_tensor(xap, xap, pc[:, :, b * HS:(b + 1) * HS],
                                        op=Alu.add)

        # ================= phase C + tail, in two half-batches =================
        # Each half (8 chunks = one batch) runs L1 -> L2 -> log_cosh tail;
        # the second half's matmuls overlap the first half's scalar/vector
        # tail so the PE never sits idle for long.
        inv_d = 1.0 / D
        zcur = [None] * NT
        # layer-2 outputs land in one resident tensor (reusing the Toeplitz
        # bank's SBUF slot, dead after phase B) so the tail can fence on
        # a whole half at once; the bf16 exp buffer reuses V's slot.
        Z2 = consts.tile([128, NT, D], BF, tag="tb", name="Z2")
        awl = vpool.tile([128, NT, D], BF, tag="V")
        stats_t = {}
        for l in range(L):
            stats_t[l] = dict(
                sums=statp.tile([128, NT], F32, tag=f"sum{l}", name="sums"),
                ssq=statp.tile([128, NT], F32, tag=f"ssq{l}", name="ssq"),
                muA=statp.tile([128, NT], F32, tag=f"mu{l}", name="muA"),
                rsF=statp.tile([128, NT], F32, tag=f"rs{l}", name="rsF"),
                biasF=statp.tile([128, NT], F32, tag=f"bi{l}", name="biasF"),
            )
        fence = statp.tile([128, 2], F32, tag="fence")

        for l in range(L):
                fast = lnf_uniform[l] is not None
                stt = stats_t[l]
                sums, ssq = stt["sums"], stt["ssq"]
                muA, rsF, biasF = stt["muA"], stt["rsF"], stt["biasF"]
                for g0 in range(0, NT, AB):
                    for t in range(g0, g0 + AB):
                        b, jc = divmod(t, NJ)
                        if l == 0:
                            # bf16 copy of x1 feeds the 2-byte XBAR transpose
                            src = x1p.tile([128, D], BF, tag="x1b", name="x1b")
                            nc.vector.tensor_copy(src[:], X[:, b, jc, :])
                            src = src[:]
                        else:
                            src = zcur[t][:]
                        zdt = dtp.tile([128, DC, 128], BF, tag="udt")
                        nc.sync.dma_start_transpose(zdt[:], src)
                        pf = ps_mm.tile([128, NJ, 128], F32, tag="mm")
                        pff = pf[:].rearrange("p a b -> p (a b)")
                        for c in range(DC):
                            nc.tensor.matmul(pff[:, 0:512],
                                             zdt[:, c, :],
                                             wf_s[:, l, c, 0:512],
                                             start=(c == 0), stop=(c == DC - 1))
                            nc.tensor.matmul(pff[:, 512:D],
                                             zdt[:, c, :],
                                             wf_s[:, l, c, 512:D],
                                             start=(c == 0), stop=(c == DC - 1))
                        if bf_nonzero[l]:
                            nc.vector.tensor_tensor(pff[:, 0:D], pff[:, 0:D],
                                                    bft[l][:], op=Alu.add)
                        y = acts.tile([128, D], BF, tag="acts")
                        nc.scalar.activation(y[:], pff[:, 0:D], Act.Copy,
                                             accum_out=sums[:, t:t + 1])
                        scr = scrp.tile([128, D], BF, tag="scr")
                        nc.vector.scalar_tensor_tensor(
                            scr[:], y[:], 0.0, y[:], op0=Alu.add, op1=Alu.mult,
                            accum_out=ssq[:, t:t + 1])
                        zcur[t] = y
                    # group epilogue: var -> rstd (DVE Newton) -> Silu
                    g = slice(g0, g0 + AB)
                    nc.vector.tensor_scalar(muA[:, g], sums[:, g], inv_d, None,
                                            op0=Alu.mult)
                    m2 = stat.tile([128, 16], F32, tag="m2", name="m2")[:, 0:AB]
                    nc.vector.tensor_scalar(m2, ssq[:, g], inv_d, None,
                                            op0=Alu.mult)
                    var = stat.tile([128, 16], F32, tag="var", name="var")[:, 0:AB]
                    nc.vector.scalar_tensor_tensor(var, muA[:, g], -1.0,
                                                   muA[:, g], op0=Alu.mult,
                                                   op1=Alu.mult)
                    nc.vector.tensor_tensor(var, m2, var, op=Alu.add)
                    _newton_rsqrt(nc, stat, rsF[:, g], var, magict, AB)
                    if fast:
                        cs, cb = lnf_uniform[l]
                        if cs != 1.0:
                            nc.vector.tensor_scalar(rsF[:, g], rsF[:, g],
                                                    float(cs), None,
                                                    op0=Alu.mult)
                        nc.vector.scalar_tensor_tensor(biasF[:, g], muA[:, g],
                                                       -1.0, rsF[:, g],
                                                       op0=Alu.mult,
                                                       op1=Alu.mult)
                        if cb != 0.0:
                            nc.vector.tensor_scalar(biasF[:, g], biasF[:, g],
                                                    float(cb), None,
                                                    op0=Alu.add)
                        for t in range(g0, g0 + AB):
                            y = zcur[t]
                            dst = y[:] if l == 0 else Z2[:, t, :]
                            nc.scalar.activation(dst, y[:], Act.Silu,
                                                 bias=biasF[:, t:t + 1],
                                                 scale=rsF[:, t:t + 1])
                            if l == 1:
                                zcur[t] = None
                    else:
                        for t in range(g0, g0 + AB):
                            y = zcur[t]
                            tmp = acts.tile([128, D], BF, tag="acts")
                            nc.vector.tensor_scalar(tmp[:], y[:],
                                                    muA[:, t:t + 1],
                                                    rsF[:, t:t + 1],
                                                    op0=Alu.subtract,
                                                    op1=Alu.mult)
                            nc.vector.tensor_tensor(tmp[:], tmp[:], lnfst[l][:],
                                                    op=Alu.mult)
                            dst = tmp[:] if l == 0 else Z2[:, t, :]
                            nc.vector.tensor_tensor(dst, tmp[:], lnfbt[l][:],
                                                    op=Alu.add)
                            nc.scalar.activation(dst, dst, Act.Silu,
                                                 bias=zerot[:])
                            zcur[t] = tmp if l == 0 else None

        # ---- tail, in two halves: log_cosh(w) = |w| + log1p(exp(-2|w|)) - ln2
        # Half 1's exp/ln overlaps layer 2's back half on the PE; per-chunk
        # output DMAs overlap the transfers with the remaining compute.
        for h0 in range(0, NT, NJ):
            hh = h0 // NJ
            for t in range(h0, h0 + NJ):
                b, jc = divmod(t, NJ)
                xt = X[:, b, jc, :]
                nc.vector.tensor_tensor(xt, xt, Z2[:, t, :], op=Alu.add)
                nc.vector.scalar_tensor_tensor(xt, xt, -1.0, xt,
                                               op0=Alu.mult, op1=Alu.max)
            # fence: a zero [128,1] that depends on every layer-2 Silu of
            # this half; used as the Exp bias so the scheduler cannot
            # interleave tail Exps between Silus (activation-table thrash).
            fscr = stat.tile([128, NJ], F32, tag="fscr", name="fscr")
            nc.vector.tensor_scalar(fscr[:], Z2[:, h0:h0 + NJ, 0], 0.0, 0.0,
                                    op0=Alu.mult, op1=Alu.mult,
                                    accum_out=fence[:, hh:hh + 1])
            for t in range(h0, h0 + NJ):
                b, jc = divmod(t, NJ)
                nc.scalar.activation(awl[:, t, :], X[:, b, jc, :], Act.Exp,
                                     bias=fence[:, hh:hh + 1], scale=-2.0)
            for t in range(h0, h0 + NJ):
                nc.scalar.activation(awl[:, t, :], awl[:, t, :], Act.Ln,
                                     bias=onet[:], scale=1.0)
            for t in range(h0, h0 + NJ):
                b, jc = divmod(t, NJ)
                sp = wkp.tile([128, D], F32, tag="sp", name="sp", bufs=4)
                nc.vector.scalar_tensor_tensor(sp[:], awl[:, t, :],
                                               -LN2, X[:, b, jc, :],
                                               op0=Alu.add, op1=Alu.add)
                nc.scalar.dma_start(out_d[b, jc * 128:(jc + 1) * 128, :], sp[:])


def _prep(inputs):
    x = np.asarray(inputs["x"], np.float32)
    ln1_s = np.asarray(inputs["ln1_scale"], np.float32)
    ln1_b = np.asarray(inputs["ln1_bias"], np.float32)
    Wv = np.asarray(inputs["Wv"], np.float32)
    alpha = np.asarray(inputs["alpha"], np.float32)
    Wf = np.asarray(inputs["Wf"], np.float32)
    bfv = np.asarray(inputs["bf"], np.float32)
    lnf_s = np.asarray(inputs["lnf_scale"], np.float32)
    lnf_b = np.asarray(inputs["lnf_bias"], np.float32)

    Wv_flat = Wv.transpose(1, 0, 2).reshape(D, H * HS)
    Wvp = (ln1_s[:, None] * Wv_flat).astype(BF16)
    cv = (ln1_b @ Wv_flat).astype(np.float32)

    ar = alpha[:, (-np.arange(N)) % N]
    ar2 = np.concatenate([ar, ar], axis=1)
    m_ = np.arange(NJ)[:, None, None]
    p_ = np.arange(128)[None, :, None]
    f_ = np.arange(128)[None, None, :]
    T = ar2[:, N + 128 * m_ + p_ - f_]                  # [H, NJ, 128, 128]
    tbank = np.ascontiguousarray(
        T.transpose(0, 2, 1, 3).reshape(H, 128, NJ * 128)).astype(BF16)

    cv_nonzero = bool(np.any(cv))
    bf_nonzero = tuple(bool(np.any(bfv[l])) for l in range(L))
    lnf_uniform = []
    for l in range(L):
        s, bb = lnf_s[l], lnf_b[l]
        if np.all(s == s[0]) and np.all(bb == bb[0]):
            lnf_uniform.append((float(s[0]), float(bb[0])))
        else:
            lnf_uniform.append(None)
    key = (cv_nonzero, bf_nonzero, tuple(lnf_uniform))

    common = {
        "wv": np.ascontiguousarray(Wvp),
        "wf": Wf.astype(BF16),
        "tbank": tbank,
        "id32": np.eye(128, dtype=np.float32),
        "idbf": np.eye(128, dtype=BF16),
        "cv": cv,
        "bfb": bfv,
        "lnfs": lnf_s,
        "lnfb": lnf_b,
    }
    return x, key, common, (cv_nonzero, bf_nonzero, lnf_uniform)


def kernel(**inputs):
    x, key, common, flags = _prep(inputs)
    if key not in _cache:
        _cache[key] = _build(*flags)
    nc = _cache[key]
    in_maps = []
    for i in range(NCORES):
        m = dict(common)
        m["xs"] = np.ascontiguousarray(x[i * BPC:(i + 1) * BPC])
        in_maps.append(m)
    res = run_bass_kernel_spmd(nc, in_maps, core_ids=list(range(NCORES)),
                               trace=TRACE, **TRACE_KW)
    kernel.last_result = res
    out = np.empty((B, N, D), np.float32)
    for i in range(NCORES):
        out[i * BPC:(i + 1) * BPC] = res.results[i]["out"]
    return out
